# revision 13
# baseline (speedup 1.0000x reference)
"""Trainium2 Bass kernel: Gaussian-splat projection preprocessing.

Computes, for N=4M Gaussians (data-parallel over 8 NeuronCores):
  pos_view = clip((p - t) @ R^T, -100, 100)
  z        = clip(pos_view_z, 0.1, 10)
  x        = clip(pos_view_x * FX / z + W/2,  -1000, W+1000)
  y        = clip(-pos_view_y * FY / z + H/2, -1000, H+1000)
  proj     = [x, y, pos_view_z]
  cov2d    = [[min(sx^2+1e-4,1e6), 1e-6], [1e-6, min(sy^2+1e-4,1e6)]],
             s{x,y} = max(scale_{x,y}, 1e-3) * F{X,Y} / z
  colors   = clip(colors, 0, 1)
  opac     = sigmoid(opacities)
  mask     = (pos_view_z > 0.1) & (pos_view_z < 10)

`rotations`/`projmat` are unused by the reference math and never touch the device.
"""

import os
import sys
from contextlib import ExitStack

import numpy as np

for _p in ("/opt/trn_rl_repo", os.path.expanduser("~/.axon_site/_ro/trn_rl_repo")):
    if os.path.isdir(_p) and _p not in sys.path:
        sys.path.append(_p)

import concourse.bacc as bacc
import concourse.mybir as mybir
import concourse.tile as tile
from concourse.bass_utils import run_bass_kernel_spmd

dt = mybir.dt
Alu = mybir.AluOpType
Act = mybir.ActivationFunctionType

N_CORES = 8
P = 128
FX = 500.0
FY = 500.0
# Per-core tile free dims: sum=3908 -> 128*3908 = 500224 rows/core,
# 8 cores cover 4 001 792 >= 4 000 000 (pad 1792 rows on the last core).
# Small first tile so compute starts before the first full-size load lands;
# small last tile so the trailing store drain is short.
FS = (256, 896, 896, 896, 640, 324)
ROWS_PER_CORE = P * sum(FS)


def _build(viewmat, width, height, fs=FS):
    """Trace+compile the per-core Bass program. viewmat values are baked as
    immediates (they are runtime inputs of kernel(); we JIT per call)."""
    rows = P * sum(fs)
    vm = np.asarray(viewmat, np.float32)
    R = vm[:3, :3]
    t = vm[:3, 3]
    W2 = float(width) / 2.0
    H2 = float(height) / 2.0
    XHI = float(width) + 1000.0
    YHI = float(height) + 1000.0
    CS = float(np.float32(0.001) * np.float32(FX))  # max(s,.001)*FX == max(s*FX, CS)

    nc = bacc.Bacc(
        "TRN2",
        target_bir_lowering=False,
        debug=False,
        enable_asserts=False,
        num_devices=N_CORES,
    )
    f32 = dt.float32
    pos_d = nc.dram_tensor("positions", [rows, 3], f32, kind="ExternalInput").ap()
    scl_d = nc.dram_tensor("scales", [rows, 3], f32, kind="ExternalInput").ap()
    col_d = nc.dram_tensor("colors", [rows, 3], f32, kind="ExternalInput").ap()
    opa_d = nc.dram_tensor("opacities", [rows], f32, kind="ExternalInput").ap()
    pp_d = nc.dram_tensor("proj_out", [rows, 3], f32, kind="ExternalOutput").ap()
    cov_d = nc.dram_tensor("cov_out", [rows, 4], f32, kind="ExternalOutput").ap()
    colo_d = nc.dram_tensor("col_out", [rows, 3], f32, kind="ExternalOutput").ap()
    opao_d = nc.dram_tensor("opa_out", [rows], f32, kind="ExternalOutput").ap()
    msk_d = nc.dram_tensor("mask_out", [rows], dt.uint8, kind="ExternalOutput").ap()

    with tile.TileContext(nc) as tc, ExitStack() as ctx:
        inp = ctx.enter_context(tc.tile_pool(name="inp", bufs=2))
        outp = ctx.enter_context(tc.tile_pool(name="outp", bufs=3))
        tmp = ctx.enter_context(tc.tile_pool(name="tmp", bufs=2))
        cpool = ctx.enter_context(tc.tile_pool(name="consts", bufs=1))

        # per-partition bias constants for non-Copy activations
        cbias = cpool.tile([P, 2], f32, tag="cbias")
        nc.gpsimd.memset(cbias[:, 0:1], -CS)
        nc.gpsimd.memset(cbias[:, 1:2], CS)
        neg_cs, pos_cs = cbias[:, 0:1], cbias[:, 1:2]

        r0 = 0
        for F in fs:
            G = P * F
            sl = slice(r0, r0 + G)
            r0 += G

            pos = inp.tile([P, 3 * F], f32, tag="pos")
            scl = inp.tile([P, 3 * F], f32, tag="scl")
            col = inp.tile([P, 3 * F], f32, tag="col")
            opa = inp.tile([P, F], f32, tag="opa")
            nc.sync.dma_start(pos[:], pos_d[sl, :].rearrange("(p f) c -> p (f c)", p=P))
            nc.sync.dma_start(scl[:], scl_d[sl, :].rearrange("(p f) c -> p (f c)", p=P))
            nc.sync.dma_start(col[:], col_d[sl, :].rearrange("(p f) c -> p (f c)", p=P))
            nc.sync.dma_start(opa[:], opa_d[sl].rearrange("(p f) -> p f", p=P))

            pp = outp.tile([P, 3 * F], f32, tag="pp")
            cov = outp.tile([P, 4 * F], f32, tag="cov")
            msk = outp.tile([P, F], dt.uint8, tag="msk")

            pv0 = tmp.tile([P, F], f32, tag="pv0")
            pv1 = tmp.tile([P, F], f32, tag="pv1")
            pv2 = tmp.tile([P, F], f32, tag="pv2")
            z = tmp.tile([P, F], f32, tag="z")
            rz = tmp.tile([P, F], f32, tag="rz")
            q0 = tmp.tile([P, F], f32, tag="q0")

            px, py, pz = pos[:, 0::3], pos[:, 1::3], pos[:, 2::3]
            s0, s1 = scl[:, 0::3], scl[:, 1::3]

            # --- view transform: pv_j = ((pz*Rj2 + bj) + py*Rj1) + px*Rj0,
            # with bj = -(R[j] . t) folded into the ACT affine.
            b = [-float(np.float32(R[j]) @ np.float32(t)) for j in range(3)]
            for j, pvj in enumerate((pv0, pv1, pv2)):
                nc.scalar.activation(
                    pvj[:], pz, Act.Copy, scale=float(R[j, 2]), bias=b[j]
                )
            for j, pvj in enumerate((pv0, pv1, pv2)):
                nc.vector.scalar_tensor_tensor(
                    pvj[:], py, float(R[j, 1]), pvj[:], Alu.mult, Alu.add
                )
            for j, pvj in enumerate((pv0, pv1, pv2)):
                nc.vector.scalar_tensor_tensor(
                    pvj[:], px, float(R[j, 0]), pvj[:], Alu.mult, Alu.add
                )

            # --- proj z column, z, clips
            nc.vector.tensor_scalar(
                pp[:, 2::3], pv2[:], -100.0, 100.0, Alu.max, Alu.min
            )
            nc.vector.tensor_scalar(z[:], pv2[:], 0.1, 10.0, Alu.max, Alu.min)
            nc.vector.tensor_scalar(pv0[:], pv0[:], -100.0, 100.0, Alu.max, Alu.min)
            nc.vector.tensor_scalar(pv1[:], pv1[:], -100.0, 100.0, Alu.max, Alu.min)

            # --- rz = 1/z (z in [0.1,10]; ~2 ULP), scratch reuses pv2
            nc.vector.reciprocal_approx_accurate(rz[:], z[:], scratch=pv2[:])

            # --- x,y screen coords
            nc.vector.tensor_tensor(pv0[:], pv0[:], rz[:], Alu.mult)
            nc.scalar.activation(pv0[:], pv0[:], Act.Copy, scale=FX, bias=W2)
            nc.vector.tensor_scalar(
                pp[:, 0::3], pv0[:], -1000.0, XHI, Alu.max, Alu.min
            )
            nc.vector.tensor_tensor(pv1[:], pv1[:], rz[:], Alu.mult)
            nc.scalar.activation(pv1[:], pv1[:], Act.Copy, scale=-FY, bias=H2)
            nc.vector.tensor_scalar(
                pp[:, 1::3], pv1[:], -1000.0, YHI, Alu.max, Alu.min
            )

            # --- cov diagonal: c = min((max(s*F,CS)*rz)^2 + 1e-4, 1e6)
            nc.scalar.activation(rz[:], rz[:], Act.Square)  # rz <- rz^2
            q1 = pv2  # reuse (dead after reciprocal scratch)
            for s_in, q, cov_col in ((s0, q0, cov[:, 0::4]), (s1, q1, cov[:, 3::4])):
                nc.scalar.activation(q[:], s_in, Act.Relu, scale=FX, bias=neg_cs)
                nc.scalar.activation(q[:], q[:], Act.Square, bias=pos_cs)
                nc.vector.tensor_tensor(q[:], q[:], rz[:], Alu.mult)
                nc.vector.tensor_scalar(cov_col, q[:], 1e-4, 1e6, Alu.add, Alu.min)
            cov3 = cov[:].rearrange("p (f c) -> p f c", c=4)
            nc.gpsimd.memset(cov3[:, :, 1:3], 1e-6)

            # --- colors / opacity (in place; colors on gpsimd to unload DVE)
            nc.gpsimd.tensor_scalar(col[:], col[:], 0.0, 1.0, Alu.max, Alu.min)
            nc.scalar.activation(opa[:], opa[:], Act.Sigmoid)

            # --- mask = (z > 0.1) & (z < 10): boundary-exact vs reference
            g = pv1  # reuse (dead after y column written)
            nc.vector.tensor_scalar(g[:], z[:], 0.1, None, Alu.is_gt)
            nc.vector.scalar_tensor_tensor(
                msk[:], z[:], 10.0, g[:], Alu.is_lt, Alu.logical_and
            )

            # --- stores (SWDGE ring via gpsimd: own queue, keeps ACT/SP free)
            nc.gpsimd.dma_start(pp_d[sl, :].rearrange("(p f) c -> p (f c)", p=P), pp[:])
            nc.gpsimd.dma_start(cov_d[sl, :].rearrange("(p f) c -> p (f c)", p=P), cov[:])
            nc.gpsimd.dma_start(colo_d[sl, :].rearrange("(p f) c -> p (f c)", p=P), col[:])
            nc.gpsimd.dma_start(opao_d[sl].rearrange("(p f) -> p f", p=P), opa[:])
            nc.gpsimd.dma_start(msk_d[sl].rearrange("(p f) -> p f", p=P), msk[:])

    nc.compile()
    return nc


_CACHE = {}


def _get_nc(viewmat, width, height, fs=FS):
    key = (np.asarray(viewmat, np.float32).tobytes(), float(width), float(height), fs)
    if key not in _CACHE:
        _CACHE[key] = _build(viewmat, width, height, fs)
    return _CACHE[key]


def _shard(arr, core, rows):
    """Rows [core*rows, (core+1)*rows) of arr, zero-padded past the end."""
    n = arr.shape[0]
    lo = core * rows
    hi = lo + rows
    if hi <= n:
        return arr[lo:hi]
    out = np.zeros((rows,) + arr.shape[1:], arr.dtype)
    if lo < n:
        out[: n - lo] = arr[lo:]
    return out


def kernel(
    positions,
    scales,
    rotations,
    colors,
    opacities,
    viewmat,
    projmat,
    width,
    height,
    _trace=False,
):
    positions = np.ascontiguousarray(np.asarray(positions, np.float32))
    scales = np.ascontiguousarray(np.asarray(scales, np.float32))
    colors = np.ascontiguousarray(np.asarray(colors, np.float32))
    opacities = np.ascontiguousarray(np.asarray(opacities, np.float32))
    n = positions.shape[0]

    nc = _get_nc(viewmat, width, height)
    in_maps = [
        {
            "positions": _shard(positions, c, ROWS_PER_CORE),
            "scales": _shard(scales, c, ROWS_PER_CORE),
            "colors": _shard(colors, c, ROWS_PER_CORE),
            "opacities": _shard(opacities, c, ROWS_PER_CORE),
        }
        for c in range(N_CORES)
    ]
    res = run_bass_kernel_spmd(nc, in_maps, list(range(N_CORES)), trace=_trace)
    outs = res.results

    def cat(name):
        return np.concatenate([outs[c][name] for c in range(N_CORES)], axis=0)[:n]

    proj = cat("proj_out")
    cov = cat("cov_out").reshape(n, 2, 2)
    col = cat("col_out")
    opa = cat("opa_out")
    msk = cat("mask_out").astype(bool)
    if _trace:
        return (proj, cov, col, opa, msk), res
    return proj, cov, col, opa, msk


# revision 14
# speedup vs baseline: 2.2185x; 2.2185x over previous
"""Trainium2 Bass kernel: Gaussian-splat projection preprocessing.

Computes, for N=4M Gaussians (data-parallel over 8 NeuronCores):
  pos_view = clip((p - t) @ R^T, -100, 100)
  z        = clip(pos_view_z, 0.1, 10)
  x        = clip(pos_view_x * FX / z + W/2,  -1000, W+1000)
  y        = clip(-pos_view_y * FY / z + H/2, -1000, H+1000)
  proj     = [x, y, pos_view_z]
  cov2d    = [[min(sx^2+1e-4,1e6), 1e-6], [1e-6, min(sy^2+1e-4,1e6)]],
             s{x,y} = max(scale_{x,y}, 1e-3) * F{X,Y} / z
  colors   = clip(colors, 0, 1)
  opac     = sigmoid(opacities)
  mask     = (pos_view_z > 0.1) & (pos_view_z < 10)

`rotations`/`projmat` are unused by the reference math and never touch the device.
"""

import os
import sys
from contextlib import ExitStack

import numpy as np

for _p in ("/opt/trn_rl_repo", os.path.expanduser("~/.axon_site/_ro/trn_rl_repo")):
    if os.path.isdir(_p) and _p not in sys.path:
        sys.path.append(_p)

import concourse.bacc as bacc
import concourse.mybir as mybir
import concourse.tile as tile
from concourse.bass_utils import run_bass_kernel_spmd

dt = mybir.dt
Alu = mybir.AluOpType
Act = mybir.ActivationFunctionType

N_CORES = 8
P = 128
FX = 500.0
FY = 500.0
# Per-core tile free dims: sum=3908 -> 128*3908 = 500224 rows/core,
# 8 cores cover 4 001 792 >= 4 000 000 (pad 1792 rows on the last core).
# Small first tile so compute starts before the first full-size load lands;
# small last tile so the trailing store drain is short.
FS = (256, 896, 896, 896, 640, 324)
ROWS_PER_CORE = P * sum(FS)


def _build(viewmat, width, height, fs=FS):
    """Trace+compile the per-core Bass program. viewmat values are baked as
    immediates (they are runtime inputs of kernel(); we JIT per call)."""
    rows = P * sum(fs)
    vm = np.asarray(viewmat, np.float32)
    R = vm[:3, :3]
    t = vm[:3, 3]
    W2 = float(width) / 2.0
    H2 = float(height) / 2.0
    XHI = float(width) + 1000.0
    YHI = float(height) + 1000.0
    CS = float(np.float32(0.001) * np.float32(FX))  # max(s,.001)*FX == max(s*FX, CS)

    nc = bacc.Bacc(
        "TRN2",
        target_bir_lowering=False,
        debug=False,
        enable_asserts=False,
        num_devices=N_CORES,
    )
    f32 = dt.float32
    pos_d = nc.dram_tensor("positions", [rows, 3], f32, kind="ExternalInput").ap()
    scl_d = nc.dram_tensor("scales", [rows, 3], f32, kind="ExternalInput").ap()
    col_d = nc.dram_tensor("colors", [rows, 3], f32, kind="ExternalInput").ap()
    opa_d = nc.dram_tensor("opacities", [rows], f32, kind="ExternalInput").ap()
    pp_d = nc.dram_tensor("proj_out", [rows, 3], f32, kind="ExternalOutput").ap()
    cov_d = nc.dram_tensor("cov_out", [rows, 4], f32, kind="ExternalOutput").ap()
    colo_d = nc.dram_tensor("col_out", [rows, 3], f32, kind="ExternalOutput").ap()
    opao_d = nc.dram_tensor("opa_out", [rows], f32, kind="ExternalOutput").ap()
    msk_d = nc.dram_tensor("mask_out", [rows], dt.uint8, kind="ExternalOutput").ap()

    with tile.TileContext(nc) as tc, ExitStack() as ctx:
        inp = ctx.enter_context(tc.tile_pool(name="inp", bufs=2))
        outp = ctx.enter_context(tc.tile_pool(name="outp", bufs=3))
        tmp = ctx.enter_context(tc.tile_pool(name="tmp", bufs=2))
        cpool = ctx.enter_context(tc.tile_pool(name="consts", bufs=1))

        # per-partition bias constants for non-Copy activations
        cbias = cpool.tile([P, 2], f32, tag="cbias")
        nc.gpsimd.memset(cbias[:, 0:1], -CS)
        nc.gpsimd.memset(cbias[:, 1:2], CS)
        neg_cs, pos_cs = cbias[:, 0:1], cbias[:, 1:2]

        r0 = 0
        for F in fs:
            G = P * F
            sl = slice(r0, r0 + G)
            r0 += G

            pos = inp.tile([P, 3 * F], f32, tag="pos")
            scl = inp.tile([P, 3 * F], f32, tag="scl")
            col = inp.tile([P, 3 * F], f32, tag="col")
            opa = inp.tile([P, F], f32, tag="opa")
            nc.sync.dma_start(pos[:], pos_d[sl, :].rearrange("(p f) c -> p (f c)", p=P))
            nc.sync.dma_start(scl[:], scl_d[sl, :].rearrange("(p f) c -> p (f c)", p=P))
            nc.sync.dma_start(col[:], col_d[sl, :].rearrange("(p f) c -> p (f c)", p=P))
            nc.sync.dma_start(opa[:], opa_d[sl].rearrange("(p f) -> p f", p=P))

            pp = outp.tile([P, 3 * F], f32, tag="pp")
            cov = outp.tile([P, 4 * F], f32, tag="cov")
            msk = outp.tile([P, F], dt.uint8, tag="msk")

            pv0 = tmp.tile([P, F], f32, tag="pv0")
            pv1 = tmp.tile([P, F], f32, tag="pv1")
            pv2 = tmp.tile([P, F], f32, tag="pv2")
            z = tmp.tile([P, F], f32, tag="z")
            rz = tmp.tile([P, F], f32, tag="rz")
            q0 = tmp.tile([P, F], f32, tag="q0")

            px, py, pz = pos[:, 0::3], pos[:, 1::3], pos[:, 2::3]
            s0, s1 = scl[:, 0::3], scl[:, 1::3]

            # --- view transform: pv_j = ((pz*Rj2 + bj) + py*Rj1) + px*Rj0,
            # with bj = -(R[j] . t) folded into the ACT affine.
            b = [-float(np.float32(R[j]) @ np.float32(t)) for j in range(3)]
            for j, pvj in enumerate((pv0, pv1, pv2)):
                nc.scalar.activation(
                    pvj[:], pz, Act.Copy, scale=float(R[j, 2]), bias=b[j]
                )
            for j, pvj in enumerate((pv0, pv1, pv2)):
                nc.vector.scalar_tensor_tensor(
                    pvj[:], py, float(R[j, 1]), pvj[:], Alu.mult, Alu.add
                )
            for j, pvj in enumerate((pv0, pv1, pv2)):
                nc.vector.scalar_tensor_tensor(
                    pvj[:], px, float(R[j, 0]), pvj[:], Alu.mult, Alu.add
                )

            # --- proj z column, z, clips
            nc.vector.tensor_scalar(
                pp[:, 2::3], pv2[:], -100.0, 100.0, Alu.max, Alu.min
            )
            nc.vector.tensor_scalar(z[:], pv2[:], 0.1, 10.0, Alu.max, Alu.min)
            nc.vector.tensor_scalar(pv0[:], pv0[:], -100.0, 100.0, Alu.max, Alu.min)
            nc.vector.tensor_scalar(pv1[:], pv1[:], -100.0, 100.0, Alu.max, Alu.min)

            # --- rz = 1/z (z in [0.1,10]; ~2 ULP), scratch reuses pv2
            nc.vector.reciprocal_approx_accurate(rz[:], z[:], scratch=pv2[:])

            # --- x,y screen coords
            nc.vector.tensor_tensor(pv0[:], pv0[:], rz[:], Alu.mult)
            nc.scalar.activation(pv0[:], pv0[:], Act.Copy, scale=FX, bias=W2)
            nc.vector.tensor_scalar(
                pp[:, 0::3], pv0[:], -1000.0, XHI, Alu.max, Alu.min
            )
            nc.vector.tensor_tensor(pv1[:], pv1[:], rz[:], Alu.mult)
            nc.scalar.activation(pv1[:], pv1[:], Act.Copy, scale=-FY, bias=H2)
            nc.vector.tensor_scalar(
                pp[:, 1::3], pv1[:], -1000.0, YHI, Alu.max, Alu.min
            )

            # --- cov diagonal: c = min((max(s*F,CS)*rz)^2 + 1e-4, 1e6)
            nc.scalar.activation(rz[:], rz[:], Act.Square)  # rz <- rz^2
            q1 = pv2  # reuse (dead after reciprocal scratch)
            for s_in, q, cov_col in ((s0, q0, cov[:, 0::4]), (s1, q1, cov[:, 3::4])):
                nc.scalar.activation(q[:], s_in, Act.Relu, scale=FX, bias=neg_cs)
                nc.scalar.activation(q[:], q[:], Act.Square, bias=pos_cs)
                nc.vector.tensor_tensor(q[:], q[:], rz[:], Alu.mult)
                nc.vector.tensor_scalar(cov_col, q[:], 1e-4, 1e6, Alu.add, Alu.min)
            cov3 = cov[:].rearrange("p (f c) -> p f c", c=4)
            nc.gpsimd.memset(cov3[:, :, 1:3], 1e-6)

            # --- colors / opacity (in place; colors on gpsimd to unload DVE)
            nc.vector.tensor_scalar(col[:], col[:], 0.0, 1.0, Alu.max, Alu.min)
            nc.scalar.activation(opa[:], opa[:], Act.Sigmoid)

            # --- mask = (z > 0.1) & (z < 10): boundary-exact vs reference
            g = pv1  # reuse (dead after y column written)
            nc.vector.tensor_scalar(g[:], z[:], 0.1, None, Alu.is_gt)
            nc.vector.scalar_tensor_tensor(
                msk[:], z[:], 10.0, g[:], Alu.is_lt, Alu.logical_and
            )

            # --- stores (SWDGE ring via gpsimd: own queue, keeps ACT/SP free)
            nc.gpsimd.dma_start(pp_d[sl, :].rearrange("(p f) c -> p (f c)", p=P), pp[:])
            nc.gpsimd.dma_start(cov_d[sl, :].rearrange("(p f) c -> p (f c)", p=P), cov[:])
            nc.gpsimd.dma_start(colo_d[sl, :].rearrange("(p f) c -> p (f c)", p=P), col[:])
            nc.gpsimd.dma_start(opao_d[sl].rearrange("(p f) -> p f", p=P), opa[:])
            nc.gpsimd.dma_start(msk_d[sl].rearrange("(p f) -> p f", p=P), msk[:])

    nc.compile()
    return nc


_CACHE = {}


def _get_nc(viewmat, width, height, fs=FS):
    key = (np.asarray(viewmat, np.float32).tobytes(), float(width), float(height), fs)
    if key not in _CACHE:
        _CACHE[key] = _build(viewmat, width, height, fs)
    return _CACHE[key]


def _shard(arr, core, rows):
    """Rows [core*rows, (core+1)*rows) of arr, zero-padded past the end."""
    n = arr.shape[0]
    lo = core * rows
    hi = lo + rows
    if hi <= n:
        return arr[lo:hi]
    out = np.zeros((rows,) + arr.shape[1:], arr.dtype)
    if lo < n:
        out[: n - lo] = arr[lo:]
    return out


def kernel(
    positions,
    scales,
    rotations,
    colors,
    opacities,
    viewmat,
    projmat,
    width,
    height,
    _trace=False,
):
    positions = np.ascontiguousarray(np.asarray(positions, np.float32))
    scales = np.ascontiguousarray(np.asarray(scales, np.float32))
    colors = np.ascontiguousarray(np.asarray(colors, np.float32))
    opacities = np.ascontiguousarray(np.asarray(opacities, np.float32))
    n = positions.shape[0]

    nc = _get_nc(viewmat, width, height)
    in_maps = [
        {
            "positions": _shard(positions, c, ROWS_PER_CORE),
            "scales": _shard(scales, c, ROWS_PER_CORE),
            "colors": _shard(colors, c, ROWS_PER_CORE),
            "opacities": _shard(opacities, c, ROWS_PER_CORE),
        }
        for c in range(N_CORES)
    ]
    res = run_bass_kernel_spmd(nc, in_maps, list(range(N_CORES)), trace=_trace)
    outs = res.results

    def cat(name):
        return np.concatenate([outs[c][name] for c in range(N_CORES)], axis=0)[:n]

    proj = cat("proj_out")
    cov = cat("cov_out").reshape(n, 2, 2)
    col = cat("col_out")
    opa = cat("opa_out")
    msk = cat("mask_out").astype(bool)
    if _trace:
        return (proj, cov, col, opa, msk), res
    return proj, cov, col, opa, msk


# revision 33
# speedup vs baseline: 2.2320x; 1.0061x over previous
"""Trainium2 Bass kernel: Gaussian-splat projection preprocessing.

Computes, for N=4M Gaussians (data-parallel over 8 NeuronCores):
  pos_view = clip((p - t) @ R^T, -100, 100)
  z        = clip(pos_view_z, 0.1, 10)
  x        = clip(pos_view_x * FX / z + W/2,  -1000, W+1000)
  y        = clip(-pos_view_y * FY / z + H/2, -1000, H+1000)
  proj     = [x, y, pos_view_z]
  cov2d    = [[min(sx^2+1e-4,1e6), 1e-6], [1e-6, min(sy^2+1e-4,1e6)]],
             s{x,y} = max(scale_{x,y}, 1e-3) * F{X,Y} / z
  colors   = clip(colors, 0, 1)
  opac     = sigmoid(opacities)
  mask     = (pos_view_z > 0.1) & (pos_view_z < 10)

`rotations`/`projmat` are unused by the reference math and never touch the device.
"""

import os
import sys
from contextlib import ExitStack

import numpy as np

for _p in ("/opt/trn_rl_repo", os.path.expanduser("~/.axon_site/_ro/trn_rl_repo")):
    if os.path.isdir(_p) and _p not in sys.path:
        sys.path.append(_p)

import concourse.bacc as bacc
import concourse.mybir as mybir
import concourse.tile as tile
from concourse.bass_utils import run_bass_kernel_spmd

dt = mybir.dt
Alu = mybir.AluOpType
Act = mybir.ActivationFunctionType

N_CORES = 8
P = 128
FX = 500.0
FY = 500.0
# Per-core tile free dims: sum=3908 -> 128*3908 = 500224 rows/core,
# 8 cores cover 4 001 792 >= 4 000 000 (pad 1792 rows on the last core).
# Small first tile so compute starts before the first full-size load lands;
# small last tile so the trailing store drain is short.
FS = (256, 896, 896, 896, 640, 324)
ROWS_PER_CORE = P * sum(FS)
RECIP_FAST = False  # 1 custom DVE op (~51 ULP) vs 2 (~2 ULP); no speed gain measured
LOAD_SPLIT = False  # col/opa loads on the ACT HWDGE ring instead of SP
STORE_SPLIT = False  # pp/opa stores on the ACT ring instead of SWDGE
PACKED = False  # host packs [pos, s0, s1, opa] into one (rows, 6) input; measured slower


BUFS = (2, 3)  # (inp, outp)


def _build(viewmat, width, height, fs=FS):
    """Trace+compile the per-core Bass program. viewmat values are baked as
    immediates (they are runtime inputs of kernel(); we JIT per call)."""
    rows = P * sum(fs)
    vm = np.asarray(viewmat, np.float32)
    R = vm[:3, :3]
    t = vm[:3, 3]
    W2 = float(width) / 2.0
    H2 = float(height) / 2.0
    XHI = float(width) + 1000.0
    YHI = float(height) + 1000.0
    CS = float(np.float32(0.001) * np.float32(FX))  # max(s,.001)*FX == max(s*FX, CS)

    nc = bacc.Bacc(
        "TRN2",
        target_bir_lowering=False,
        debug=False,
        enable_asserts=False,
        num_devices=N_CORES,
    )
    f32 = dt.float32
    if PACKED:
        pk_d = nc.dram_tensor("packed", [rows, 6], f32, kind="ExternalInput").ap()
    else:
        pos_d = nc.dram_tensor("positions", [rows, 3], f32, kind="ExternalInput").ap()
        scl_d = nc.dram_tensor("scales", [rows, 3], f32, kind="ExternalInput").ap()
        opa_d = nc.dram_tensor("opacities", [rows], f32, kind="ExternalInput").ap()
    col_d = nc.dram_tensor("colors", [rows, 3], f32, kind="ExternalInput").ap()
    pp_d = nc.dram_tensor("proj_out", [rows, 3], f32, kind="ExternalOutput").ap()
    cov_d = nc.dram_tensor("cov_out", [rows, 4], f32, kind="ExternalOutput").ap()
    colo_d = nc.dram_tensor("col_out", [rows, 3], f32, kind="ExternalOutput").ap()
    opao_d = nc.dram_tensor("opa_out", [rows], f32, kind="ExternalOutput").ap()
    msk_d = nc.dram_tensor("mask_out", [rows], dt.uint8, kind="ExternalOutput").ap()

    with tile.TileContext(nc) as tc, ExitStack() as ctx:
        inp = ctx.enter_context(tc.tile_pool(name="inp", bufs=BUFS[0]))
        outp = ctx.enter_context(tc.tile_pool(name="outp", bufs=BUFS[1]))
        tmp = ctx.enter_context(tc.tile_pool(name="tmp", bufs=2))
        cpool = ctx.enter_context(tc.tile_pool(name="consts", bufs=1))

        # per-partition bias constants for non-Copy activations
        cbias = cpool.tile([P, 2], f32, tag="cbias")
        nc.gpsimd.memset(cbias[:, 0:1], -CS)
        nc.gpsimd.memset(cbias[:, 1:2], CS)
        neg_cs, pos_cs = cbias[:, 0:1], cbias[:, 1:2]

        r0 = 0
        for F in fs:
            G = P * F
            sl = slice(r0, r0 + G)
            r0 += G

            ld2 = nc.scalar if LOAD_SPLIT else nc.sync
            col = inp.tile([P, 3 * F], f32, tag="col")
            if PACKED:
                pk = inp.tile([P, 6 * F], f32, tag="pk")
                nc.sync.dma_start(
                    pk[:], pk_d[sl, :].rearrange("(p f) c -> p (f c)", p=P)
                )
            else:
                pos = inp.tile([P, 3 * F], f32, tag="pos")
                scl = inp.tile([P, 3 * F], f32, tag="scl")
                opa = inp.tile([P, F], f32, tag="opa")
                nc.sync.dma_start(
                    pos[:], pos_d[sl, :].rearrange("(p f) c -> p (f c)", p=P)
                )
                nc.sync.dma_start(
                    scl[:], scl_d[sl, :].rearrange("(p f) c -> p (f c)", p=P)
                )
                ld2.dma_start(opa[:], opa_d[sl].rearrange("(p f) -> p f", p=P))
            ld2.dma_start(col[:], col_d[sl, :].rearrange("(p f) c -> p (f c)", p=P))

            pp = outp.tile([P, 3 * F], f32, tag="pp")
            cov = outp.tile([P, 4 * F], f32, tag="cov")
            msk = outp.tile([P, F], dt.uint8, tag="msk")
            if PACKED:
                opa_o = outp.tile([P, F], f32, tag="opa_o")

            pv0 = tmp.tile([P, F], f32, tag="pv0")
            pv1 = tmp.tile([P, F], f32, tag="pv1")
            pv2 = tmp.tile([P, F], f32, tag="pv2")
            z = tmp.tile([P, F], f32, tag="z")
            rz = tmp.tile([P, F], f32, tag="rz")
            q0 = tmp.tile([P, F], f32, tag="q0")

            if PACKED:
                px, py, pz = pk[:, 0::6], pk[:, 1::6], pk[:, 2::6]
                s0, s1 = pk[:, 3::6], pk[:, 4::6]
            else:
                px, py, pz = pos[:, 0::3], pos[:, 1::3], pos[:, 2::3]
                s0, s1 = scl[:, 0::3], scl[:, 1::3]

            # --- view transform: pv_j = ((pz*Rj2 + bj) + py*Rj1) + px*Rj0,
            # with bj = -(R[j] . t) folded into the ACT affine.
            b = [-float(np.float32(R[j]) @ np.float32(t)) for j in range(3)]
            for j, pvj in enumerate((pv0, pv1, pv2)):
                nc.scalar.activation(
                    pvj[:], pz, Act.Copy, scale=float(R[j, 2]), bias=b[j]
                )
            for j, pvj in enumerate((pv0, pv1, pv2)):
                nc.vector.scalar_tensor_tensor(
                    pvj[:], py, float(R[j, 1]), pvj[:], Alu.mult, Alu.add
                )
            for j, pvj in enumerate((pv0, pv1, pv2)):
                nc.vector.scalar_tensor_tensor(
                    pvj[:], px, float(R[j, 0]), pvj[:], Alu.mult, Alu.add
                )

            # --- proj z column, z, clips
            nc.vector.tensor_scalar(
                pp[:, 2::3], pv2[:], -100.0, 100.0, Alu.max, Alu.min
            )
            nc.vector.tensor_scalar(z[:], pv2[:], 0.1, 10.0, Alu.max, Alu.min)
            nc.vector.tensor_scalar(pv0[:], pv0[:], -100.0, 100.0, Alu.max, Alu.min)
            nc.vector.tensor_scalar(pv1[:], pv1[:], -100.0, 100.0, Alu.max, Alu.min)

            # --- rz = 1/z (z in [0.1,10])
            if RECIP_FAST:
                nc.vector.reciprocal_approx_fast(rz[:], z[:])
            else:
                nc.vector.reciprocal_approx_accurate(rz[:], z[:], scratch=pv2[:])

            # --- x,y screen coords
            nc.vector.tensor_tensor(pv0[:], pv0[:], rz[:], Alu.mult)
            nc.scalar.activation(pv0[:], pv0[:], Act.Copy, scale=FX, bias=W2)
            nc.vector.tensor_scalar(
                pp[:, 0::3], pv0[:], -1000.0, XHI, Alu.max, Alu.min
            )
            nc.vector.tensor_tensor(pv1[:], pv1[:], rz[:], Alu.mult)
            nc.scalar.activation(pv1[:], pv1[:], Act.Copy, scale=-FY, bias=H2)
            nc.vector.tensor_scalar(
                pp[:, 1::3], pv1[:], -1000.0, YHI, Alu.max, Alu.min
            )

            # --- cov diagonal: c = min((max(s*F,CS)*rz)^2 + 1e-4, 1e6)
            nc.scalar.activation(rz[:], rz[:], Act.Square)  # rz <- rz^2
            q1 = pv2  # reuse (dead after reciprocal scratch)
            for s_in, q, cov_col in ((s0, q0, cov[:, 0::4]), (s1, q1, cov[:, 3::4])):
                nc.scalar.activation(q[:], s_in, Act.Relu, scale=FX, bias=neg_cs)
                nc.scalar.activation(q[:], q[:], Act.Square, bias=pos_cs)
                nc.vector.tensor_tensor(q[:], q[:], rz[:], Alu.mult)
                nc.vector.tensor_scalar(cov_col, q[:], 1e-4, 1e6, Alu.add, Alu.min)
            cov3 = cov[:].rearrange("p (f c) -> p f c", c=4)
            nc.gpsimd.memset(cov3[:, :, 1:3], 1e-6)

            # --- colors / opacity
            nc.vector.tensor_scalar(col[:], col[:], 0.0, 1.0, Alu.max, Alu.min)
            if PACKED:
                nc.scalar.activation(opa_o[:], pk[:, 5::6], Act.Sigmoid)
            else:
                nc.scalar.activation(opa[:], opa[:], Act.Sigmoid)

            # --- mask = (z > 0.1) & (z < 10): boundary-exact vs reference
            g = pv1  # reuse (dead after y column written)
            nc.vector.tensor_scalar(g[:], z[:], 0.1, None, Alu.is_gt)
            nc.vector.scalar_tensor_tensor(
                msk[:], z[:], 10.0, g[:], Alu.is_lt, Alu.logical_and
            )

            # --- stores (SWDGE ring via gpsimd: own queue, keeps ACT/SP free)
            st2 = nc.scalar if STORE_SPLIT else nc.gpsimd
            st2.dma_start(pp_d[sl, :].rearrange("(p f) c -> p (f c)", p=P), pp[:])
            nc.gpsimd.dma_start(cov_d[sl, :].rearrange("(p f) c -> p (f c)", p=P), cov[:])
            nc.gpsimd.dma_start(colo_d[sl, :].rearrange("(p f) c -> p (f c)", p=P), col[:])
            st2.dma_start(
                opao_d[sl].rearrange("(p f) -> p f", p=P),
                opa_o[:] if PACKED else opa[:],
            )
            nc.gpsimd.dma_start(msk_d[sl].rearrange("(p f) -> p f", p=P), msk[:])

    nc.compile()
    return nc


_CACHE = {}


def _get_nc(viewmat, width, height, fs=FS):
    key = (
        np.asarray(viewmat, np.float32).tobytes(),
        float(width),
        float(height),
        fs,
        PACKED,
        RECIP_FAST,
        BUFS,
    )
    if key not in _CACHE:
        _CACHE[key] = _build(viewmat, width, height, fs)
    return _CACHE[key]


def _shard(arr, core, rows):
    """Rows [core*rows, (core+1)*rows) of arr, zero-padded past the end."""
    n = arr.shape[0]
    lo = core * rows
    hi = lo + rows
    if hi <= n:
        return arr[lo:hi]
    out = np.zeros((rows,) + arr.shape[1:], arr.dtype)
    if lo < n:
        out[: n - lo] = arr[lo:]
    return out


def kernel(
    positions,
    scales,
    rotations,
    colors,
    opacities,
    viewmat,
    projmat,
    width,
    height,
    _trace=False,
):
    positions = np.ascontiguousarray(np.asarray(positions, np.float32))
    scales = np.ascontiguousarray(np.asarray(scales, np.float32))
    colors = np.ascontiguousarray(np.asarray(colors, np.float32))
    opacities = np.ascontiguousarray(np.asarray(opacities, np.float32))
    n = positions.shape[0]

    nc = _get_nc(viewmat, width, height)
    if PACKED:
        pk = np.empty((n, 6), np.float32)
        pk[:, 0:3] = positions
        pk[:, 3] = scales[:, 0]
        pk[:, 4] = scales[:, 1]
        pk[:, 5] = opacities
        in_maps = [
            {
                "packed": _shard(pk, c, ROWS_PER_CORE),
                "colors": _shard(colors, c, ROWS_PER_CORE),
            }
            for c in range(N_CORES)
        ]
    else:
        in_maps = [
            {
                "positions": _shard(positions, c, ROWS_PER_CORE),
                "scales": _shard(scales, c, ROWS_PER_CORE),
                "colors": _shard(colors, c, ROWS_PER_CORE),
                "opacities": _shard(opacities, c, ROWS_PER_CORE),
            }
            for c in range(N_CORES)
        ]
    res = run_bass_kernel_spmd(nc, in_maps, list(range(N_CORES)), trace=_trace)
    outs = res.results

    def cat(name):
        return np.concatenate([outs[c][name] for c in range(N_CORES)], axis=0)[:n]

    proj = cat("proj_out")
    cov = cat("cov_out").reshape(n, 2, 2)
    col = cat("col_out")
    opa = cat("opa_out")
    msk = cat("mask_out").astype(bool)
    if _trace:
        return (proj, cov, col, opa, msk), res
    return proj, cov, col, opa, msk
